# revision 8
# baseline (speedup 1.0000x reference)
"""Trainium2 Bass kernel for nn_ExtendedSelfAttention (B=4, S=2048, D=4096, H=1).

With n_heads=1 the softmax is over a size-1 axis, so attention weights are
exactly 1.0 and the module reduces to:

    out = (value @ Wv.T + bv) @ Wo.T + bo
        = value @ (Wo @ Wv).T + (Wo @ bv + bo)

(query/key/Wq/Wk never affect the output.) Since there are 8192 tokens but
only 4096 features, composing the weights first cuts total FLOPs by 25%:
computing Wc^T = (Wo @ Wv)^T costs one 4096^3 GEMM (sharded 8 ways), after
which only ONE token GEMM is needed instead of two.

Sharding (no collectives):
  phase A: core c computes Wc^T[:, c*512:(c+1)*512]   (1024 bf16 matmuls)
  phase B: core c computes out[:, c*512:(c+1)*512] for ALL 8192 tokens
Output is column-sharded; the host concatenates. The fused bias
bias2 = Wo @ bv + bo is computed exactly on the host and added in phase B.

Mixed precision in phase B: the first KF=1280 contraction columns run as
fp8-e4m3 DoubleRow matmuls (two k-tiles per MM -> ~2x MAC rate), the
remaining 2816 columns in bf16, all accumulating into the same PSUM group.
Measured on the real inputs this lands at rel_err ~1.8e-2 vs the 2e-2 gate
(pure bf16 is 3.0e-3; pure fp8 would be 3.8e-2).  Wc is computed scaled by
SCL=512 (host scales Wo) so its e4m3 image clears the subnormal zone; the
descale is fused into the PSUM->SBUF copy on the otherwise-idle ScalarE.
"""

import numpy as np

B, S, D = 4, 2048, 4096
N_CORES = 8
TOK = B * S           # 8192 tokens
P = 128
KO = D // P           # 32 contraction tiles (phase A)
GBLK = D // N_CORES   # 512 output columns per core
TT = TOK // P         # 64 token tiles

KF = 1280             # phase-B contraction columns done in fp8 DoubleRow
KP = KF // 256        # 5 DoubleRow pair-tiles
KOB = (D - KF) // P   # 22 bf16 k-tiles in phase B
SCL = 512.0           # Wc scale so e4m3 quantization clears subnormals

_CACHED = {}


def _build_nc():
    import concourse.bass as bass  # noqa: F401  (registers engine builders)
    import concourse.tile as tile
    from concourse import bacc, mybir

    bf16 = mybir.dt.bfloat16
    fp8 = mybir.dt.float8e4
    f32 = mybir.dt.float32
    DR = mybir.MatmulPerfMode.DoubleRow
    Copy = mybir.ActivationFunctionType.Copy

    nc = bacc.Bacc("TRN2", target_bir_lowering=False, debug=False,
                   num_devices=N_CORES)

    # wv[m, p, fo, c2] = Wv[fo*128+p, m*128+c2]      (lhsT tiles for phase A)
    wv = nc.declare_dram_parameter("wv", [KO, P, KO, P], bf16, isOutput=False)
    # woT[p, fo, g] = SCL * Wo[cg0+g, fo*128+p]      (rhs for phase A, per-core)
    woT = nc.declare_dram_parameter("woT", [P, KO, GBLK], bf16, isOutput=False)
    # xt[tt, p, ko, tc] = x[tt*128+tc, KF+ko*128+p]  (bf16 lhsT for phase B)
    xt = nc.declare_dram_parameter("xt", [TT, P, KOB, P], bf16, isOutput=False)
    # xt8[tt, p, kk*256+i*128+tc... ] fp8 pairs: [tt][p][kk][i][tc]
    xt8 = nc.declare_dram_parameter("xt8", [TT, P, KP * 2 * P], fp8,
                                    isOutput=False)
    b2 = nc.declare_dram_parameter("b2", [P, GBLK], f32, isOutput=False)
    out = nc.declare_dram_parameter("out", [TOK, GBLK], f32, isOutput=True)

    with tile.TileContext(nc) as tc:
        with tc.tile_pool(name="const", bufs=1) as const_pool, \
             tc.tile_pool(name="wot", bufs=1) as wot_pool, \
             tc.tile_pool(name="wct", bufs=1) as wct_pool, \
             tc.tile_pool(name="wvp", bufs=5) as wv_pool, \
             tc.tile_pool(name="xtp", bufs=4) as xt_pool, \
             tc.tile_pool(name="x8p", bufs=4) as x8_pool, \
             tc.tile_pool(name="psum", bufs=8, space="PSUM") as psum_pool, \
             tc.tile_pool(name="stage", bufs=4) as stage_pool:
            wot_sb = wot_pool.tile([P, KO, GBLK], bf16)
            wct_sb = wct_pool.tile([P, KOB, GBLK], bf16)
            wct8_sb = wct_pool.tile([P, KP, 2, GBLK], fp8)

            # Prewarm the PE during the DMA ramp: the HAM clock gate needs
            # ~3.4us of sustained matmul activity to lift the PE from 1.2 to
            # 2.4 GHz.  Small-N matmuls burn minimal wall time per rep while
            # counting as activity, so the flip happens as early as possible.
            warm_lhs = const_pool.tile([P, P], bf16, tag="warm_lhs")
            warm_rhs = const_pool.tile([P, P], bf16, tag="warm_rhs")
            nc.gpsimd.memset(warm_lhs[:], 0.0)
            nc.gpsimd.memset(warm_rhs[:], 0.0)
            N_WARM = 20
            dps = psum_pool.tile([P, P], f32, tag="ps")
            for i in range(N_WARM):
                nc.tensor.matmul(dps[:], warm_lhs[:], warm_rhs[:],
                                 start=(i == 0), stop=(i == N_WARM - 1))

            # ---- phase A: Wc^T slice = Wv.T-contracted with Wo^T slice ----
            # Startup is DMA-ring-paced: mA=0 needs all of wot (4MB) + wv0
            # (1MB) before it completes, so wot streams on the SCALAR HWDGE
            # ring (free from ~6us) in fa-order while wv streams in parallel
            # on the SYNC ring.  First matmul needs only wot[0:2]+wv0[0:2].
            CH = [(0, 2), (2, 8), (8, 16), (16, 24), (24, 32)]

            def dma_wv(dst, m, chunks):
                for lo, hi in chunks:
                    nc.sync.dma_start(out=dst[:, lo:hi, :],
                                      in_=wv[m][:, lo:hi, :])

            nc.scalar.dma_start(out=wot_sb[:, 0:2, :], in_=woT[:, 0:2, :])
            for lo, hi in CH[1:]:
                nc.scalar.dma_start(out=wot_sb[:, lo:hi, :],
                                    in_=woT[:, lo:hi, :])
            wv_pre = []
            for m in range(4):
                wv_t = wv_pool.tile([P, KO, P], bf16, tag="wv")
                dma_wv(wv_t, m, CH)
                wv_pre.append(wv_t)

            b2_t = const_pool.tile([P, GBLK], f32)
            nc.scalar.dma_start(out=b2_t[:], in_=b2[:])

            for mA in range(KO):
                if mA < 4:
                    wv_t = wv_pre[mA]
                else:
                    wv_t = wv_pool.tile([P, KO, P], bf16, tag="wv")
                    nc.sync.dma_start(out=wv_t[:], in_=wv[mA])
                ps = psum_pool.tile([P, GBLK], f32)
                for fA in range(KO):
                    nc.tensor.matmul(
                        ps[:], wv_t[:, fA, :], wot_sb[:, fA, :],
                        start=(fA == 0), stop=(fA == KO - 1),
                    )
                if mA < 2 * KP:
                    nc.vector.tensor_copy(wct8_sb[:, mA // 2, mA % 2, :],
                                          ps[:])
                else:
                    nc.vector.tensor_copy(wct_sb[:, mA - 2 * KP, :], ps[:])

            # ---- phase B: out slice = x @ Wc^T slice (+ bias2) ----
            for tt in range(TT):
                x8_t = x8_pool.tile([P, KP, 2, P], fp8)
                nc.sync.dma_start(out=x8_t[:], in_=xt8[tt])
                xt_t = xt_pool.tile([P, KOB, P], bf16)
                nc.sync.dma_start(out=xt_t[:], in_=xt[tt])
                ps = psum_pool.tile([P, GBLK], f32)
                for kk in range(KP):
                    nc.tensor.matmul(
                        ps[:], x8_t[:, kk, :, :], wct8_sb[:, kk, :, :],
                        start=(kk == 0), stop=False, perf_mode=DR,
                    )
                for k in range(KOB):
                    nc.tensor.matmul(
                        ps[:], xt_t[:, k, :], wct_sb[:, k, :],
                        start=False, stop=(k == KOB - 1),
                    )
                st = stage_pool.tile([P, GBLK], f32)
                nc.scalar.activation(st[:], ps[:], Copy, scale=1.0 / SCL)
                nc.vector.tensor_add(st[:], st[:], b2_t[:])
                nc.sync.dma_start(
                    out=out[tt * P:(tt + 1) * P, :], in_=st[:])
    nc.compile()
    return nc


def _get_nc():
    if "nc" not in _CACHED:
        _CACHED["nc"] = _build_nc()
    return _CACHED["nc"]


def _prep_inputs(value, Wv, bv, Wo, bo):
    import ml_dtypes
    bf16 = ml_dtypes.bfloat16
    e4m3 = ml_dtypes.float8_e4m3

    x = np.asarray(value, np.float32).reshape(TOK, D)
    Wv = np.asarray(Wv, np.float32)
    Wo = np.asarray(Wo, np.float32)
    bv = np.asarray(bv, np.float32)
    bo = np.asarray(bo, np.float32)

    # xt[tt, p, ko, tc] = x[tt*128+tc, KF+ko*128+p]   (bf16 k-tiles)
    xt = np.ascontiguousarray(
        x[:, KF:].reshape(TT, P, KOB, P).transpose(0, 3, 2, 1)).astype(bf16)
    # xt8[tt, p, kk, i, tc] = x[tt*128+tc, kk*256+i*128+p]  (fp8 pairs)
    xt8 = np.ascontiguousarray(
        x[:, :KF].reshape(TT, P, KP, 2, P).transpose(0, 4, 2, 3, 1)
    ).astype(e4m3).reshape(TT, P, KP * 2 * P)
    # wv_p[m, p, fo, c2] = Wv[fo*128+p, m*128+c2]
    wv_p = np.ascontiguousarray(
        Wv.reshape(KO, P, KO, P).transpose(2, 1, 0, 3)).astype(bf16)
    # woT_full[c][p, fo, g] = SCL * Wo[c*GBLK+g, fo*128+p]
    woT_full = (Wo * SCL).reshape(N_CORES, GBLK, KO, P).transpose(0, 3, 2, 1)

    bias2 = (Wo.astype(np.float64) @ bv.astype(np.float64)
             + bo.astype(np.float64)).astype(np.float32)

    in_maps = []
    for c in range(N_CORES):
        b2_c = np.ascontiguousarray(np.broadcast_to(
            bias2[c * GBLK:(c + 1) * GBLK][None, :], (P, GBLK)))
        in_maps.append({
            "xt": xt,
            "xt8": xt8,
            "wv": wv_p,
            "woT": np.ascontiguousarray(woT_full[c]).astype(bf16),
            "b2": b2_c,
        })
    return in_maps


def _run(in_maps, trace=False):
    from concourse.bass_utils import run_bass_kernel_spmd
    nc = _get_nc()
    res = run_bass_kernel_spmd(nc, in_maps, list(range(N_CORES)), trace=trace)
    return res


def kernel(**inputs):
    in_maps = _prep_inputs(inputs["value"], inputs["Wv"], inputs["bv"],
                           inputs["Wo"], inputs["bo"])
    res = _run(in_maps, trace=False)
    out = np.empty((TOK, D), np.float32)
    for c in range(N_CORES):
        out[:, c * GBLK:(c + 1) * GBLK] = res.results[c]["out"]
    return out.reshape(B, S, D)


# revision 13
# speedup vs baseline: 1.0044x; 1.0044x over previous
"""Trainium2 Bass kernel for nn_ExtendedSelfAttention (B=4, S=2048, D=4096, H=1).

With n_heads=1 the softmax is over a size-1 axis, so attention weights are
exactly 1.0 and the module reduces to:

    out = (value @ Wv.T + bv) @ Wo.T + bo
        = value @ (Wo @ Wv).T + (Wo @ bv + bo)

(query/key/Wq/Wk never affect the output.) Since there are 8192 tokens but
only 4096 features, composing the weights first cuts total FLOPs by 25%:
computing Wc^T = (Wo @ Wv)^T costs one 4096^3 GEMM (sharded 8 ways), after
which only ONE token GEMM is needed instead of two.

Sharding (no collectives):
  phase A: core c computes Wc^T[:, c*512:(c+1)*512]   (1024 bf16 matmuls)
  phase B: core c computes out[:, c*512:(c+1)*512] for ALL 8192 tokens
Output is column-sharded; the host concatenates. The fused bias
bias2 = Wo @ bv + bo is computed exactly on the host and added in phase B.

Mixed precision in phase B: the first KF=1280 contraction columns run as
fp8-e4m3 DoubleRow matmuls (two k-tiles per MM -> ~2x MAC rate), the
remaining 2816 columns in bf16, all accumulating into the same PSUM group.
Measured on the real inputs this lands at rel_err ~1.8e-2 vs the 2e-2 gate
(pure bf16 is 3.0e-3; pure fp8 would be 3.8e-2).  Wc is computed scaled by
SCL=512 (host scales Wo) so its e4m3 image clears the subnormal zone; the
descale is fused into the PSUM->SBUF copy on the otherwise-idle ScalarE.
"""

import numpy as np

B, S, D = 4, 2048, 4096
N_CORES = 8
TOK = B * S           # 8192 tokens
P = 128
KO = D // P           # 32 contraction tiles (phase A)
GBLK = D // N_CORES   # 512 output columns per core
TT = TOK // P         # 64 token tiles

KF = 1280             # phase-B contraction columns done in fp8 DoubleRow
KP = KF // 256        # 5 DoubleRow pair-tiles
KOB = (D - KF) // P   # 22 bf16 k-tiles in phase B
SCL = 512.0           # Wc scale so e4m3 quantization clears subnormals

_CACHED = {}


def _build_nc():
    import concourse.bass as bass  # noqa: F401  (registers engine builders)
    import concourse.tile as tile
    from concourse import bacc, mybir

    bf16 = mybir.dt.bfloat16
    fp8 = mybir.dt.float8e4
    f32 = mybir.dt.float32
    DR = mybir.MatmulPerfMode.DoubleRow
    Copy = mybir.ActivationFunctionType.Copy

    nc = bacc.Bacc("TRN2", target_bir_lowering=False, debug=False,
                   num_devices=N_CORES)

    # wv[m, p, fo, c2] = Wv[fo*128+p, m*128+c2]      (lhsT tiles for phase A)
    wv = nc.declare_dram_parameter("wv", [KO, P, KO, P], bf16, isOutput=False)
    # woT[p, fo, g] = SCL * Wo[cg0+g, fo*128+p]      (rhs for phase A, per-core)
    woT = nc.declare_dram_parameter("woT", [P, KO, GBLK], bf16, isOutput=False)
    # xt[tt, p, ko, tc] = x[tt*128+tc, KF+ko*128+p]  (bf16 lhsT for phase B)
    xt = nc.declare_dram_parameter("xt", [TT, P, KOB, P], bf16, isOutput=False)
    # xt8[tt, p, kk*256+i*128+tc... ] fp8 pairs: [tt][p][kk][i][tc]
    xt8 = nc.declare_dram_parameter("xt8", [TT, P, KP * 2 * P], fp8,
                                    isOutput=False)
    b2 = nc.declare_dram_parameter("b2", [P, GBLK], f32, isOutput=False)
    out = nc.declare_dram_parameter("out", [TOK, GBLK], f32, isOutput=True)

    with tile.TileContext(nc) as tc:
        with tc.tile_pool(name="const", bufs=1) as const_pool, \
             tc.tile_pool(name="wot", bufs=1) as wot_pool, \
             tc.tile_pool(name="wct", bufs=1) as wct_pool, \
             tc.tile_pool(name="wvp", bufs=12) as wv_pool, \
             tc.tile_pool(name="xtp", bufs=4) as xt_pool, \
             tc.tile_pool(name="x8p", bufs=4) as x8_pool, \
             tc.tile_pool(name="psum", bufs=8, space="PSUM") as psum_pool, \
             tc.tile_pool(name="stage", bufs=4) as stage_pool:
            wot_sb = wot_pool.tile([P, KO, GBLK], bf16)
            wct_sb = wct_pool.tile([P, KOB, GBLK], bf16)
            wct8_sb = wct_pool.tile([P, KP, 2, GBLK], fp8)

            # Prewarm the PE during the DMA ramp: the HAM clock gate needs
            # ~3.4us of sustained matmul activity to lift the PE from 1.2 to
            # 2.4 GHz.  Small-N matmuls burn minimal wall time per rep while
            # counting as activity, so the flip happens as early as possible.
            warm_lhs = const_pool.tile([P, P], bf16, tag="warm_lhs")
            warm_rhs = const_pool.tile([P, P], bf16, tag="warm_rhs")
            nc.gpsimd.memset(warm_lhs[:], 0.0)
            nc.gpsimd.memset(warm_rhs[:], 0.0)
            N_WARM = 30
            dps = psum_pool.tile([P, P], f32, tag="ps")
            for i in range(N_WARM):
                nc.tensor.matmul(dps[:], warm_lhs[:], warm_rhs[:],
                                 start=(i == 0), stop=(i == N_WARM - 1))

            # ---- phase A: Wc^T slice = Wv.T-contracted with Wo^T slice ----
            # Startup is paced by the raw DMA bandwidth ramp (~0.3 GB/us for
            # the first ~20us), not trigger latency: completing any single
            # mA group needs ALL of wot (4MB).  So the first BLK=8 mA groups
            # run interleaved fo-octet-major across 8 PSUM banks: each 1MB
            # wot octet feeds 64 matmuls (8 groups x 8 fa), dropping the
            # startup byte demand to ~0.22 GB/us so the PE never starves and
            # the HAM clock gate flips ~10us earlier.  wot streams on the
            # SCALAR HWDGE ring in fa-order; wv on the SYNC ring.
            BLK = 8
            OCT = KO // 8    # 4 fo-octets

            def wct_store(mA, ps):
                if mA < 2 * KP:
                    nc.vector.tensor_copy(wct8_sb[:, mA // 2, mA % 2, :],
                                          ps[:])
                else:
                    nc.vector.tensor_copy(wct_sb[:, mA - 2 * KP, :], ps[:])

            nc.scalar.dma_start(out=wot_sb[:, 0:2, :], in_=woT[:, 0:2, :])
            nc.scalar.dma_start(out=wot_sb[:, 2:8, :], in_=woT[:, 2:8, :])
            for o in range(1, OCT):
                nc.scalar.dma_start(out=wot_sb[:, o * 8:(o + 1) * 8, :],
                                    in_=woT[:, o * 8:(o + 1) * 8, :])
            wv_blk = []
            for m in range(BLK):
                wv_t = wv_pool.tile([P, KO, P], bf16, tag="wv")
                wv_blk.append(wv_t)
            for o in range(OCT):
                for m in range(BLK):
                    nc.sync.dma_start(out=wv_blk[m][:, o * 8:(o + 1) * 8, :],
                                      in_=wv[m][:, o * 8:(o + 1) * 8, :])

            b2_t = const_pool.tile([P, GBLK], f32)
            nc.scalar.dma_start(out=b2_t[:], in_=b2[:])

            ps_blk = []
            for _m in range(BLK):
                ps = psum_pool.tile([P, GBLK], f32)
                ps_blk.append(ps)
            for o in range(OCT):
                for m in range(BLK):
                    for fA in range(o * 8, (o + 1) * 8):
                        nc.tensor.matmul(
                            ps_blk[m][:], wv_blk[m][:, fA, :],
                            wot_sb[:, fA, :],
                            start=(fA == 0), stop=(fA == KO - 1),
                        )
            for m in range(BLK):
                wct_store(m, ps_blk[m])

            for mA in range(BLK, KO):
                wv_t = wv_pool.tile([P, KO, P], bf16, tag="wv")
                nc.sync.dma_start(out=wv_t[:], in_=wv[mA])
                ps = psum_pool.tile([P, GBLK], f32)
                for fA in range(KO):
                    nc.tensor.matmul(
                        ps[:], wv_t[:, fA, :], wot_sb[:, fA, :],
                        start=(fA == 0), stop=(fA == KO - 1),
                    )
                wct_store(mA, ps)

            # ---- phase B: out slice = x @ Wc^T slice (+ bias2) ----
            for tt in range(TT):
                x8_t = x8_pool.tile([P, KP, 2, P], fp8)
                nc.sync.dma_start(out=x8_t[:], in_=xt8[tt])
                xt_t = xt_pool.tile([P, KOB, P], bf16)
                nc.sync.dma_start(out=xt_t[:], in_=xt[tt])
                ps = psum_pool.tile([P, GBLK], f32)
                for kk in range(KP):
                    nc.tensor.matmul(
                        ps[:], x8_t[:, kk, :, :], wct8_sb[:, kk, :, :],
                        start=(kk == 0), stop=False, perf_mode=DR,
                    )
                for k in range(KOB):
                    nc.tensor.matmul(
                        ps[:], xt_t[:, k, :], wct_sb[:, k, :],
                        start=False, stop=(k == KOB - 1),
                    )
                st = stage_pool.tile([P, GBLK], f32)
                nc.scalar.activation(st[:], ps[:], Copy, scale=1.0 / SCL)
                nc.vector.tensor_add(st[:], st[:], b2_t[:])
                nc.sync.dma_start(
                    out=out[tt * P:(tt + 1) * P, :], in_=st[:])
    nc.compile()
    return nc


def _get_nc():
    if "nc" not in _CACHED:
        _CACHED["nc"] = _build_nc()
    return _CACHED["nc"]


def _prep_inputs(value, Wv, bv, Wo, bo):
    import ml_dtypes
    bf16 = ml_dtypes.bfloat16
    e4m3 = ml_dtypes.float8_e4m3

    x = np.asarray(value, np.float32).reshape(TOK, D)
    Wv = np.asarray(Wv, np.float32)
    Wo = np.asarray(Wo, np.float32)
    bv = np.asarray(bv, np.float32)
    bo = np.asarray(bo, np.float32)

    # xt[tt, p, ko, tc] = x[tt*128+tc, KF+ko*128+p]   (bf16 k-tiles)
    xt = np.ascontiguousarray(
        x[:, KF:].reshape(TT, P, KOB, P).transpose(0, 3, 2, 1)).astype(bf16)
    # xt8[tt, p, kk, i, tc] = x[tt*128+tc, kk*256+i*128+p]  (fp8 pairs)
    xt8 = np.ascontiguousarray(
        x[:, :KF].reshape(TT, P, KP, 2, P).transpose(0, 4, 2, 3, 1)
    ).astype(e4m3).reshape(TT, P, KP * 2 * P)
    # wv_p[m, p, fo, c2] = Wv[fo*128+p, m*128+c2]
    wv_p = np.ascontiguousarray(
        Wv.reshape(KO, P, KO, P).transpose(2, 1, 0, 3)).astype(bf16)
    # woT_full[c][p, fo, g] = SCL * Wo[c*GBLK+g, fo*128+p]
    woT_full = (Wo * SCL).reshape(N_CORES, GBLK, KO, P).transpose(0, 3, 2, 1)

    bias2 = (Wo.astype(np.float64) @ bv.astype(np.float64)
             + bo.astype(np.float64)).astype(np.float32)

    in_maps = []
    for c in range(N_CORES):
        b2_c = np.ascontiguousarray(np.broadcast_to(
            bias2[c * GBLK:(c + 1) * GBLK][None, :], (P, GBLK)))
        in_maps.append({
            "xt": xt,
            "xt8": xt8,
            "wv": wv_p,
            "woT": np.ascontiguousarray(woT_full[c]).astype(bf16),
            "b2": b2_c,
        })
    return in_maps


def _run(in_maps, trace=False):
    from concourse.bass_utils import run_bass_kernel_spmd
    nc = _get_nc()
    res = run_bass_kernel_spmd(nc, in_maps, list(range(N_CORES)), trace=trace)
    return res


def kernel(**inputs):
    in_maps = _prep_inputs(inputs["value"], inputs["Wv"], inputs["bv"],
                           inputs["Wo"], inputs["bo"])
    res = _run(in_maps, trace=False)
    out = np.empty((TOK, D), np.float32)
    for c in range(N_CORES):
        out[:, c * GBLK:(c + 1) * GBLK] = res.results[c]["out"]
    return out.reshape(B, S, D)


# revision 21
# speedup vs baseline: 1.0096x; 1.0052x over previous
"""Trainium2 Bass kernel for nn_ExtendedSelfAttention (B=4, S=2048, D=4096, H=1).

With n_heads=1 the softmax is over a size-1 axis, so attention weights are
exactly 1.0 and the module reduces to:

    out = (value @ Wv.T + bv) @ Wo.T + bo
        = value @ (Wo @ Wv).T + (Wo @ bv + bo)

(query/key/Wq/Wk never affect the output.) Since there are 8192 tokens but
only 4096 features, composing the weights first cuts total FLOPs by 25%:
computing Wc^T = (Wo @ Wv)^T costs one 4096^3 GEMM (sharded 8 ways), after
which only ONE token GEMM is needed instead of two.

Sharding (no collectives):
  phase A: core c computes Wc^T[:, c*512:(c+1)*512]   (1024 bf16 matmuls)
  phase B: core c computes out[:, c*512:(c+1)*512] for ALL 8192 tokens
Output is column-sharded; the host concatenates. The fused bias
bias2 = Wo @ bv + bo is computed exactly on the host and added in phase B.

Mixed precision in phase B: the first KF=1280 contraction columns run as
fp8-e4m3 DoubleRow matmuls (two k-tiles per MM -> ~2x MAC rate), the
remaining 2816 columns in bf16, all accumulating into the same PSUM group.
Measured on the real inputs this lands at rel_err ~1.8e-2 vs the 2e-2 gate
(pure bf16 is 3.0e-3; pure fp8 would be 3.8e-2).  Wc is computed scaled by
SCL=512 (host scales Wo) so its e4m3 image clears the subnormal zone; the
descale is fused into the PSUM->SBUF copy on the otherwise-idle ScalarE.
"""

import numpy as np

B, S, D = 4, 2048, 4096
N_CORES = 8
TOK = B * S           # 8192 tokens
P = 128
KO = D // P           # 32 contraction tiles (phase A)
GBLK = D // N_CORES   # 512 output columns per core
TT = TOK // P         # 64 token tiles

KF = 1280             # phase-B contraction columns done in fp8 DoubleRow
KP = KF // 256        # 5 DoubleRow pair-tiles
KOB = (D - KF) // P   # 22 bf16 k-tiles in phase B
SCL = 512.0           # Wc scale so e4m3 quantization clears subnormals

_CACHED = {}


def _build_nc(with_bias):
    import concourse.bass as bass  # noqa: F401  (registers engine builders)
    import concourse.tile as tile
    from concourse import bacc, mybir

    bf16 = mybir.dt.bfloat16
    fp8 = mybir.dt.float8e4
    f32 = mybir.dt.float32
    DR = mybir.MatmulPerfMode.DoubleRow
    Copy = mybir.ActivationFunctionType.Copy

    nc = bacc.Bacc("TRN2", target_bir_lowering=False, debug=False,
                   num_devices=N_CORES)

    # wv[m, p, fo, c2] = Wv[fo*128+p, m*128+c2]      (lhsT tiles for phase A)
    wv = nc.declare_dram_parameter("wv", [KO, P, KO, P], bf16, isOutput=False)
    # woT[p, fo, g] = SCL * Wo[cg0+g, fo*128+p]      (rhs for phase A, per-core)
    woT = nc.declare_dram_parameter("woT", [P, KO, GBLK], bf16, isOutput=False)
    # xt[tt, p, ko, tc] = x[tt*128+tc, KF+ko*128+p]  (bf16 lhsT for phase B)
    xt = nc.declare_dram_parameter("xt", [TT, P, KOB, P], bf16, isOutput=False)
    # xt8[tt, p, kk*256+i*128+tc... ] fp8 pairs: [tt][p][kk][i][tc]
    xt8 = nc.declare_dram_parameter("xt8", [TT, P, KP * 2 * P], fp8,
                                    isOutput=False)
    if with_bias:
        b2 = nc.declare_dram_parameter("b2", [P, GBLK], f32, isOutput=False)
    out = nc.declare_dram_parameter("out", [TOK, GBLK], f32, isOutput=True)

    with tile.TileContext(nc) as tc:
        with tc.tile_pool(name="const", bufs=1) as const_pool, \
             tc.tile_pool(name="wot", bufs=1) as wot_pool, \
             tc.tile_pool(name="wct", bufs=1) as wct_pool, \
             tc.tile_pool(name="wvp", bufs=12) as wv_pool, \
             tc.tile_pool(name="xtp", bufs=4) as xt_pool, \
             tc.tile_pool(name="x8p", bufs=4) as x8_pool, \
             tc.tile_pool(name="psum", bufs=8, space="PSUM") as psum_pool, \
             tc.tile_pool(name="stage", bufs=4) as stage_pool:
            wot_sb = wot_pool.tile([P, KO, GBLK], bf16)
            wct_sb = wct_pool.tile([P, KOB, GBLK], bf16)
            wct8_sb = wct_pool.tile([P, KP, 2, GBLK], fp8)

            # Prewarm the PE during the DMA ramp: the HAM clock gate needs
            # ~3.4us of sustained matmul activity to lift the PE from 1.2 to
            # 2.4 GHz.  Small-N matmuls burn minimal wall time per rep while
            # counting as activity, so the flip happens as early as possible.
            warm_lhs = const_pool.tile([P, P], bf16, tag="warm_lhs")
            warm_rhs = const_pool.tile([P, P], bf16, tag="warm_rhs")
            nc.gpsimd.memset(warm_lhs[:], 0.0)
            nc.gpsimd.memset(warm_rhs[:], 0.0)
            N_WARM = 34
            dps = psum_pool.tile([P, P], f32, tag="ps")
            for i in range(N_WARM):
                nc.tensor.matmul(dps[:], warm_lhs[:], warm_rhs[:],
                                 start=(i == 0), stop=(i == N_WARM - 1))

            # ---- phase A: Wc^T slice = Wv.T-contracted with Wo^T slice ----
            # Startup is paced by the raw DMA bandwidth ramp (~0.3 GB/us for
            # the first ~20us), not trigger latency: completing any single
            # mA group needs ALL of wot (4MB).  So the first BLK=8 mA groups
            # run interleaved fo-octet-major across 8 PSUM banks: each 1MB
            # wot octet feeds 64 matmuls (8 groups x 8 fa), dropping the
            # startup byte demand to ~0.22 GB/us so the PE never starves and
            # the HAM clock gate flips ~10us earlier.  wot streams on the
            # SCALAR HWDGE ring in fa-order; wv on the SYNC ring.
            BLK = 8
            OCT = KO // 8    # 4 fo-octets

            def wct_store(mA, ps):
                if mA < 2 * KP:
                    nc.vector.tensor_copy(wct8_sb[:, mA // 2, mA % 2, :],
                                          ps[:])
                else:
                    nc.vector.tensor_copy(wct_sb[:, mA - 2 * KP, :], ps[:])

            wv_blk = []
            for m in range(BLK):
                wv_t = wv_pool.tile([P, KO, P], bf16, tag="wv")
                wv_blk.append(wv_t)

            # Interleave both rings in consumption order; wv4-7 ride the
            # scalar ring so each wave's ~3MB splits ~evenly across rings.
            def chunk(eng, dst, dsrc, lo, hi):
                eng.dma_start(out=dst[:, lo:hi, :], in_=dsrc[:, lo:hi, :])

            chunk(nc.scalar, wot_sb, woT, 0, 2)
            chunk(nc.sync, wv_blk[0], wv[0], 0, 2)
            chunk(nc.scalar, wot_sb, woT, 2, 8)
            chunk(nc.sync, wv_blk[0], wv[0], 2, 8)
            for o in range(OCT):
                for m in range(1, 4):
                    chunk(nc.sync, wv_blk[m], wv[m], o * 8, (o + 1) * 8)
                for m in range(4, BLK):
                    chunk(nc.scalar, wv_blk[m], wv[m], o * 8, (o + 1) * 8)
                if o + 1 < OCT:
                    chunk(nc.scalar, wot_sb, woT, (o + 1) * 8, (o + 2) * 8)
                    chunk(nc.sync, wv_blk[0], wv[0], (o + 1) * 8, (o + 2) * 8)

            b2_t = None
            if with_bias:
                b2_t = const_pool.tile([P, GBLK], f32)
                nc.scalar.dma_start(out=b2_t[:], in_=b2[:])

            ps_blk = []
            for _m in range(BLK):
                ps = psum_pool.tile([P, GBLK], f32)
                ps_blk.append(ps)
            for o in range(OCT):
                for m in range(BLK):
                    for fA in range(o * 8, (o + 1) * 8):
                        nc.tensor.matmul(
                            ps_blk[m][:], wv_blk[m][:, fA, :],
                            wot_sb[:, fA, :],
                            start=(fA == 0), stop=(fA == KO - 1),
                        )
            for m in range(BLK):
                wct_store(m, ps_blk[m])

            for mA in range(BLK, KO):
                wv_t = wv_pool.tile([P, KO, P], bf16, tag="wv")
                nc.sync.dma_start(out=wv_t[:], in_=wv[mA])
                ps = psum_pool.tile([P, GBLK], f32)
                for fA in range(KO):
                    nc.tensor.matmul(
                        ps[:], wv_t[:, fA, :], wot_sb[:, fA, :],
                        start=(fA == 0), stop=(fA == KO - 1),
                    )
                wct_store(mA, ps)

            # ---- phase B: out slice = x @ Wc^T slice (+ bias2) ----
            # The last token tile runs as two 256-wide PSUM groups so the
            # first half's epilogue overlaps the second half's matmuls,
            # trimming the serial drain chain at the very end.
            for tt in range(TT):
                x8_t = x8_pool.tile([P, KP, 2, P], fp8)
                nc.sync.dma_start(out=x8_t[:], in_=xt8[tt])
                xt_t = xt_pool.tile([P, KOB, P], bf16)
                nc.sync.dma_start(out=xt_t[:], in_=xt[tt])
                halves = 1 if tt < TT - 1 else 2
                gw = GBLK // halves
                for h in range(halves):
                    gs = slice(h * gw, (h + 1) * gw)
                    ps = psum_pool.tile([P, gw], f32)
                    for kk in range(KP):
                        nc.tensor.matmul(
                            ps[:], x8_t[:, kk, :, :], wct8_sb[:, kk, :, gs],
                            start=(kk == 0), stop=False, perf_mode=DR,
                        )
                    for k in range(KOB):
                        nc.tensor.matmul(
                            ps[:], xt_t[:, k, :], wct_sb[:, k, gs],
                            start=False, stop=(k == KOB - 1),
                        )
                    st = stage_pool.tile([P, gw], f32)
                    nc.scalar.activation(st[:], ps[:], Copy, scale=1.0 / SCL)
                    if with_bias:
                        nc.vector.tensor_add(st[:], st[:], b2_t[:, gs])
                    nc.sync.dma_start(
                        out=out[tt * P:(tt + 1) * P, gs], in_=st[:])
    nc.compile()
    return nc


def _get_nc(with_bias):
    key = ("nc", with_bias)
    if key not in _CACHED:
        _CACHED[key] = _build_nc(with_bias)
    return _CACHED[key]


def _prep_inputs(value, Wv, bv, Wo, bo):
    import ml_dtypes
    bf16 = ml_dtypes.bfloat16
    e4m3 = ml_dtypes.float8_e4m3

    x = np.asarray(value, np.float32).reshape(TOK, D)
    Wv = np.asarray(Wv, np.float32)
    Wo = np.asarray(Wo, np.float32)
    bv = np.asarray(bv, np.float32)
    bo = np.asarray(bo, np.float32)

    # xt[tt, p, ko, tc] = x[tt*128+tc, KF+ko*128+p]   (bf16 k-tiles)
    xt = np.ascontiguousarray(
        x[:, KF:].reshape(TT, P, KOB, P).transpose(0, 3, 2, 1)).astype(bf16)
    # xt8[tt, p, kk, i, tc] = x[tt*128+tc, kk*256+i*128+p]  (fp8 pairs)
    xt8 = np.ascontiguousarray(
        x[:, :KF].reshape(TT, P, KP, 2, P).transpose(0, 4, 2, 3, 1)
    ).astype(e4m3).reshape(TT, P, KP * 2 * P)
    # wv_p[m, p, fo, c2] = Wv[fo*128+p, m*128+c2]
    wv_p = np.ascontiguousarray(
        Wv.reshape(KO, P, KO, P).transpose(2, 1, 0, 3)).astype(bf16)
    # woT_full[c][p, fo, g] = SCL * Wo[c*GBLK+g, fo*128+p]
    woT_full = (Wo * SCL).reshape(N_CORES, GBLK, KO, P).transpose(0, 3, 2, 1)

    bias2 = (Wo.astype(np.float64) @ bv.astype(np.float64)
             + bo.astype(np.float64)).astype(np.float32)
    with_bias = bool(np.any(bias2))

    in_maps = []
    for c in range(N_CORES):
        m = {
            "xt": xt,
            "xt8": xt8,
            "wv": wv_p,
            "woT": np.ascontiguousarray(woT_full[c]).astype(bf16),
        }
        if with_bias:
            m["b2"] = np.ascontiguousarray(np.broadcast_to(
                bias2[c * GBLK:(c + 1) * GBLK][None, :], (P, GBLK)))
        in_maps.append(m)
    return in_maps, with_bias


def _run(in_maps, with_bias=False, trace=False):
    from concourse.bass_utils import run_bass_kernel_spmd
    nc = _get_nc(with_bias)
    res = run_bass_kernel_spmd(nc, in_maps, list(range(N_CORES)), trace=trace)
    return res


def kernel(**inputs):
    in_maps, with_bias = _prep_inputs(inputs["value"], inputs["Wv"],
                                      inputs["bv"], inputs["Wo"],
                                      inputs["bo"])
    res = _run(in_maps, with_bias, trace=False)
    out = np.empty((TOK, D), np.float32)
    for c in range(N_CORES):
        out[:, c * GBLK:(c + 1) * GBLK] = res.results[c]["out"]
    return out.reshape(B, S, D)
